# revision 30
# baseline (speedup 1.0000x reference)
"""BinarizeLinear Trainium2 kernel.

Computes out = x @ sign(W).T + bias for x [262144, 512], W [512, 512],
bias [512], data-parallel over 8 NeuronCores (x sharded along rows).

Strategy per core (shard = 32768 rows):
  - PE runs fp8e4m3 matmuls in DoubleRow perf mode (2 MACs/cell/cycle).
    The DoubleRow pack dimension carries a hi/lo split of x:
    slot 0 = e4m3(x) against w, slot 1 = e4m3(16*(x - hi)) against w/16,
    so one DoubleRow matmul accumulates hi*w + lo*w/16 ~= x*w at ~bf16+
    precision into fp32 PSUM, at the bf16 cycle count. sign(W) is +-1 and
    w/16 is +-2^-4 - both exact in e4m3.
  - Host prep: x shard pre-tiled+packed into per-block, per-ko contiguous
    chunks [ko][ki=128, j=2, ns, p] fp8 so every DMA read segment is one
    contiguous run per partition AND the first matmul group of a block
    only waits on a quarter of the block's bytes. Output is written bf16
    and upcast to fp32 on host.
  - Device: per block, one x DMA per ko (sync/SP HWDGE ring), 4
    accumulating DoubleRow matmuls per 128-row subtile (lhsT = x pack
    [128,2,128], rhs = w pack [128,2,512], PSUM [128 n, 512 o]),
    bias-add on DVE copying PSUM -> SBUF bf16, one out-DMA per block on
    the scalar/ACT HWDGE ring (separate ring from reads).
  - n-assignment interleaved (lhsT column p of subtile s covers row
    p*n_sub + s) so each partition's output rows are consecutive ->
    one contiguous DRAM write segment per partition per block.
  - Block sizes ramp at start/end to shorten pipeline fill/drain; ~40
    dependency-free warmup matmuls run during the DMA fill to start the
    PE HAM clock-gate ramp early.
"""

import numpy as np
import ml_dtypes

import concourse.mybir as mybir
from concourse import bacc, bass_utils
from concourse.tile import TileContext

N_CORES = 8
N_TOTAL = 262144
IN_F = 512
OUT_F = 512
N_SHARD = N_TOTAL // N_CORES  # 32768
K_BLOCKS = IN_F // 128        # 4
P = 128
J = 2                         # DoubleRow pack: hi/lo

# ramped block schedule (rows per block); sums to N_SHARD
BLOCKS = [256, 256, 512] + [1024] * 30 + [512, 256, 128, 128]
assert sum(BLOCKS) == N_SHARD

SPLIT_KO = True  # one x-DMA per ko block (finer matmul-ready granularity)

_nc_cache = None


def _build_nc():
    nc = bacc.Bacc(
        "TRN2", target_bir_lowering=False, debug=False, num_devices=N_CORES
    )
    # x pre-packed on host: per block, per ko a contiguous [128, 2*blk] chunk
    xt_d = nc.dram_tensor(
        "xt", [N_SHARD * IN_F * J], mybir.dt.float8e4, kind="ExternalInput"
    ).ap()
    wt_d = nc.dram_tensor(
        "wt", [P, K_BLOCKS, J, OUT_F], mybir.dt.float8e4, kind="ExternalInput"
    ).ap()
    b_d = nc.dram_tensor(
        "bias_bcast", [P, OUT_F], mybir.dt.float32, kind="ExternalInput"
    ).ap()
    out_d = nc.dram_tensor(
        "out", [N_SHARD, OUT_F], mybir.dt.bfloat16, kind="ExternalOutput"
    ).ap()

    with TileContext(nc) as tc:
        with (
            tc.tile_pool(name="const", bufs=1) as cpool,
            tc.tile_pool(name="xin", bufs=4) as xpool,
            tc.tile_pool(name="outp", bufs=4) as opool,
            tc.tile_pool(name="psum", bufs=7, space="PSUM") as ppool,
            tc.tile_pool(name="warm", bufs=1, space="PSUM") as wpool,
        ):
            # dependency-free dummy matmuls on a zeroed SBUF tile: they
            # schedule at engine boot and hold the PE busy so the HAM
            # clock-gate ramp starts before the first real matmul
            scratch = cpool.tile([P, P], mybir.dt.bfloat16)
            nc.gpsimd.memset(scratch[:], 0.0)
            wps = wpool.tile([P, 64], mybir.dt.float32)
            for _ in range(40):
                nc.tensor.matmul(
                    wps[:], lhsT=scratch[:], rhs=scratch[:, :64],
                    start=True, stop=True,
                )

            # constants on the ACT (write) ring so the first x-block
            # read isn't queued behind them on the SP ring
            wt_sb = cpool.tile([P, K_BLOCKS, J, OUT_F], mybir.dt.float8e4)
            nc.scalar.dma_start(wt_sb[:], wt_d[:])
            b_sb = cpool.tile([P, OUT_F], mybir.dt.float32)
            nc.scalar.dma_start(b_sb[:], b_d[:])

            off = 0
            for bi, blk in enumerate(BLOCKS):
                n_sub = blk // P
                x_sb = [
                    xpool.tile([P, J, n_sub, P], mybir.dt.float8e4,
                               tag=f"x{ko}", name=f"x{ko}")
                    for ko in range(K_BLOCKS)
                ]
                base = off * IN_F * J
                ko_sz = blk * P * J  # elements per ko chunk
                for ko in range(K_BLOCKS):
                    src = xt_d[
                        base + ko * ko_sz:base + (ko + 1) * ko_sz
                    ].rearrange("(ki f) -> ki f", ki=P)
                    # spread the early-ramp x reads across BOTH HWDGE
                    # rings (the ACT/write ring is idle until ~13us), so
                    # the first blocks aren't bound by one ring's startup
                    eng = nc.scalar if (bi < 3 and ko % 2 == 1) else nc.sync
                    eng.dma_start(
                        x_sb[ko][:].rearrange("p j s q -> p (j s q)"), src
                    )
                o_sb = opool.tile([P, n_sub, OUT_F], mybir.dt.bfloat16)
                for ns in range(n_sub):
                    ps = ppool.tile([P, OUT_F], mybir.dt.float32)
                    for ko in range(K_BLOCKS):
                        # column p covers row off + p*n_sub + ns
                        nc.tensor.matmul(
                            ps[:],
                            lhsT=x_sb[ko][:, :, ns, :],
                            rhs=wt_sb[:, ko, :, :],
                            start=(ko == 0),
                            stop=(ko == K_BLOCKS - 1),
                            perf_mode=mybir.MatmulPerfMode.DoubleRow,
                        )
                    nc.vector.tensor_add(o_sb[:, ns, :], ps[:], b_sb[:])
                # rows [off, off+blk) as [p, s, o]: row = off + p*n_sub + s
                # -> contiguous (s, o) run of n_sub KiB per partition
                dst = out_d[off:off + blk, :].rearrange(
                    "(p s) o -> p s o", s=n_sub
                )
                nc.scalar.dma_start(dst, o_sb[:])
                off += blk

    nc.finalize()
    return nc


_E4 = ml_dtypes.float8_e4m3


def _pack_x_shard(shard_f32: np.ndarray) -> np.ndarray:
    """[N_SHARD, 512] fp32 -> flat fp8 per-block [ko][ki, j, ns, p] pack."""
    chunks = []
    off = 0
    for blk in BLOCKS:
        n_sub = blk // P
        b = shard_f32[off:off + blk, :].reshape(P, n_sub, K_BLOCKS, P)
        # axes: [p, ns, ko, ki]
        hi = b.astype(_E4)
        lo = ((b - hi.astype(np.float32)) * 16.0).astype(_E4)
        pack = np.stack([hi, lo], axis=0)        # [j, p, ns, ko, ki]
        pack = pack.transpose(3, 4, 0, 2, 1)     # [ko, ki, j, ns, p]
        chunks.append(np.ascontiguousarray(pack).reshape(-1))
        off += blk
    return np.concatenate(chunks)


def kernel(x: np.ndarray, weight: np.ndarray, bias: np.ndarray, **run_kwargs):
    global _nc_cache
    if _nc_cache is None:
        _nc_cache = _build_nc()
    nc = _nc_cache

    x = np.asarray(x)
    weight = np.asarray(weight)
    bias = np.asarray(bias)

    wb = np.sign(weight.astype(np.float32)).T          # [512 i, 512 o]
    wbr = wb.reshape(K_BLOCKS, P, OUT_F)               # [ko, ki, o]
    wt = np.stack(
        [wbr.astype(_E4), (wbr / 16.0).astype(_E4)], axis=2
    )                                                  # [ko, ki, j, o]
    wt = np.ascontiguousarray(wt.transpose(1, 0, 2, 3))  # [ki, ko, j, o]
    bias_bcast = np.ascontiguousarray(
        np.broadcast_to(bias.astype(np.float32)[None, :], (P, OUT_F))
    )

    in_maps = []
    for c in range(N_CORES):
        shard = np.ascontiguousarray(
            x[c * N_SHARD:(c + 1) * N_SHARD, :], dtype=np.float32
        )
        in_maps.append(
            {"xt": _pack_x_shard(shard), "wt": wt, "bias_bcast": bias_bcast}
        )

    res = bass_utils.run_bass_kernel_spmd(
        nc, in_maps, core_ids=list(range(N_CORES)), **run_kwargs
    )
    out = np.empty((N_TOTAL, OUT_F), dtype=np.float32)
    for c in range(N_CORES):
        out[c * N_SHARD:(c + 1) * N_SHARD, :] = res.results[c]["out"].astype(
            np.float32
        )
    if run_kwargs:
        kernel.last_result = res
    return out


# revision 32
# speedup vs baseline: 1.0048x; 1.0048x over previous
"""BinarizeLinear Trainium2 kernel.

Computes out = x @ sign(W).T + bias for x [262144, 512], W [512, 512],
bias [512], data-parallel over 8 NeuronCores (x sharded along rows).

Strategy per core (shard = 32768 rows):
  - PE runs fp8e4m3 matmuls in DoubleRow perf mode (2 MACs/cell/cycle).
    The DoubleRow pack dimension carries a hi/lo split of x:
    slot 0 = e4m3(x) against w, slot 1 = e4m3(16*(x - hi)) against w/16,
    so one DoubleRow matmul accumulates hi*w + lo*w/16 ~= x*w at ~bf16+
    precision into fp32 PSUM, at the bf16 cycle count. sign(W) is +-1 and
    w/16 is +-2^-4 - both exact in e4m3.
  - Host prep: x shard pre-tiled+packed into per-block, per-ko contiguous
    chunks [ko][ki=128, j=2, ns, p] fp8 so every DMA read segment is one
    contiguous run per partition AND the first matmul group of a block
    only waits on a quarter of the block's bytes. Output is written bf16
    and upcast to fp32 on host.
  - Device: per block, one x DMA per ko (sync/SP HWDGE ring), 4
    accumulating DoubleRow matmuls per 128-row subtile (lhsT = x pack
    [128,2,128], rhs = w pack [128,2,512], PSUM [128 n, 512 o]),
    bias-add on DVE copying PSUM -> SBUF bf16, one out-DMA per block on
    the scalar/ACT HWDGE ring (separate ring from reads).
  - n-assignment interleaved (lhsT column p of subtile s covers row
    p*n_sub + s) so each partition's output rows are consecutive ->
    one contiguous DRAM write segment per partition per block.
  - Block sizes ramp at start/end to shorten pipeline fill/drain; ~40
    dependency-free warmup matmuls run during the DMA fill to start the
    PE HAM clock-gate ramp early.
"""

import numpy as np
import ml_dtypes

import concourse.mybir as mybir
from concourse import bacc, bass_utils
from concourse.tile import TileContext

N_CORES = 8
N_TOTAL = 262144
IN_F = 512
OUT_F = 512
N_SHARD = N_TOTAL // N_CORES  # 32768
K_BLOCKS = IN_F // 128        # 4
P = 128
J = 2                         # DoubleRow pack: hi/lo

# ramped block schedule (rows per block); sums to N_SHARD
BLOCKS = [256, 256, 512] + [1024] * 30 + [512, 256, 256]
assert sum(BLOCKS) == N_SHARD

SPLIT_KO = True  # one x-DMA per ko block (finer matmul-ready granularity)

_nc_cache = None


def _build_nc():
    nc = bacc.Bacc(
        "TRN2", target_bir_lowering=False, debug=False, num_devices=N_CORES
    )
    # x pre-packed on host: per block, per ko a contiguous [128, 2*blk] chunk
    xt_d = nc.dram_tensor(
        "xt", [N_SHARD * IN_F * J], mybir.dt.float8e4, kind="ExternalInput"
    ).ap()
    wt_d = nc.dram_tensor(
        "wt", [P, K_BLOCKS, J, OUT_F], mybir.dt.float8e4, kind="ExternalInput"
    ).ap()
    b_d = nc.dram_tensor(
        "bias_bcast", [P, OUT_F], mybir.dt.float32, kind="ExternalInput"
    ).ap()
    out_d = nc.dram_tensor(
        "out", [N_SHARD, OUT_F], mybir.dt.bfloat16, kind="ExternalOutput"
    ).ap()

    with TileContext(nc) as tc:
        with (
            tc.tile_pool(name="const", bufs=1) as cpool,
            tc.tile_pool(name="xin", bufs=4) as xpool,
            tc.tile_pool(name="outp", bufs=4) as opool,
            tc.tile_pool(name="psum", bufs=7, space="PSUM") as ppool,
            tc.tile_pool(name="warm", bufs=1, space="PSUM") as wpool,
        ):
            # dependency-free dummy matmuls on a zeroed SBUF tile: they
            # schedule at engine boot and hold the PE busy so the HAM
            # clock-gate ramp starts before the first real matmul
            scratch = cpool.tile([P, P], mybir.dt.bfloat16)
            nc.gpsimd.memset(scratch[:], 0.0)
            wps = wpool.tile([P, 64], mybir.dt.float32)
            for _ in range(40):
                nc.tensor.matmul(
                    wps[:], lhsT=scratch[:], rhs=scratch[:, :64],
                    start=True, stop=True,
                )

            # constants on the ACT (write) ring so the first x-block
            # read isn't queued behind them on the SP ring
            wt_sb = cpool.tile([P, K_BLOCKS, J, OUT_F], mybir.dt.float8e4)
            nc.scalar.dma_start(wt_sb[:], wt_d[:])
            b_sb = cpool.tile([P, OUT_F], mybir.dt.float32)
            nc.scalar.dma_start(b_sb[:], b_d[:])

            off = 0
            for bi, blk in enumerate(BLOCKS):
                n_sub = blk // P
                x_sb = [
                    xpool.tile([P, J, n_sub, P], mybir.dt.float8e4,
                               tag=f"x{ko}", name=f"x{ko}")
                    for ko in range(K_BLOCKS)
                ]
                base = off * IN_F * J
                ko_sz = blk * P * J  # elements per ko chunk
                for ko in range(K_BLOCKS):
                    src = xt_d[
                        base + ko * ko_sz:base + (ko + 1) * ko_sz
                    ].rearrange("(ki f) -> ki f", ki=P)
                    nc.sync.dma_start(
                        x_sb[ko][:].rearrange("p j s q -> p (j s q)"), src
                    )
                o_sb = opool.tile([P, n_sub, OUT_F], mybir.dt.bfloat16)
                for ns in range(n_sub):
                    ps = ppool.tile([P, OUT_F], mybir.dt.float32)
                    for ko in range(K_BLOCKS):
                        # column p covers row off + p*n_sub + ns
                        nc.tensor.matmul(
                            ps[:],
                            lhsT=x_sb[ko][:, :, ns, :],
                            rhs=wt_sb[:, ko, :, :],
                            start=(ko == 0),
                            stop=(ko == K_BLOCKS - 1),
                            perf_mode=mybir.MatmulPerfMode.DoubleRow,
                        )
                    nc.vector.tensor_add(o_sb[:, ns, :], ps[:], b_sb[:])
                # rows [off, off+blk) as [p, s, o]: row = off + p*n_sub + s
                # -> contiguous (s, o) run of n_sub KiB per partition
                dst = out_d[off:off + blk, :].rearrange(
                    "(p s) o -> p s o", s=n_sub
                )
                nc.scalar.dma_start(dst, o_sb[:])
                off += blk

    nc.finalize()
    return nc


_E4 = ml_dtypes.float8_e4m3


def _pack_x_shard(shard_f32: np.ndarray) -> np.ndarray:
    """[N_SHARD, 512] fp32 -> flat fp8 per-block [ko][ki, j, ns, p] pack."""
    chunks = []
    off = 0
    for blk in BLOCKS:
        n_sub = blk // P
        b = shard_f32[off:off + blk, :].reshape(P, n_sub, K_BLOCKS, P)
        # axes: [p, ns, ko, ki]
        hi = b.astype(_E4)
        lo = ((b - hi.astype(np.float32)) * 16.0).astype(_E4)
        pack = np.stack([hi, lo], axis=0)        # [j, p, ns, ko, ki]
        pack = pack.transpose(3, 4, 0, 2, 1)     # [ko, ki, j, ns, p]
        chunks.append(np.ascontiguousarray(pack).reshape(-1))
        off += blk
    return np.concatenate(chunks)


def kernel(x: np.ndarray, weight: np.ndarray, bias: np.ndarray, **run_kwargs):
    global _nc_cache
    if _nc_cache is None:
        _nc_cache = _build_nc()
    nc = _nc_cache

    x = np.asarray(x)
    weight = np.asarray(weight)
    bias = np.asarray(bias)

    wb = np.sign(weight.astype(np.float32)).T          # [512 i, 512 o]
    wbr = wb.reshape(K_BLOCKS, P, OUT_F)               # [ko, ki, o]
    wt = np.stack(
        [wbr.astype(_E4), (wbr / 16.0).astype(_E4)], axis=2
    )                                                  # [ko, ki, j, o]
    wt = np.ascontiguousarray(wt.transpose(1, 0, 2, 3))  # [ki, ko, j, o]
    bias_bcast = np.ascontiguousarray(
        np.broadcast_to(bias.astype(np.float32)[None, :], (P, OUT_F))
    )

    in_maps = []
    for c in range(N_CORES):
        shard = np.ascontiguousarray(
            x[c * N_SHARD:(c + 1) * N_SHARD, :], dtype=np.float32
        )
        in_maps.append(
            {"xt": _pack_x_shard(shard), "wt": wt, "bias_bcast": bias_bcast}
        )

    res = bass_utils.run_bass_kernel_spmd(
        nc, in_maps, core_ids=list(range(N_CORES)), **run_kwargs
    )
    out = np.empty((N_TOTAL, OUT_F), dtype=np.float32)
    for c in range(N_CORES):
        out[c * N_SHARD:(c + 1) * N_SHARD, :] = res.results[c]["out"].astype(
            np.float32
        )
    if run_kwargs:
        kernel.last_result = res
    return out


# revision 33
# speedup vs baseline: 1.0104x; 1.0055x over previous
"""BinarizeLinear Trainium2 kernel.

Computes out = x @ sign(W).T + bias for x [262144, 512], W [512, 512],
bias [512], data-parallel over 8 NeuronCores (x sharded along rows).

Strategy per core (shard = 32768 rows):
  - PE runs fp8e4m3 matmuls in DoubleRow perf mode (2 MACs/cell/cycle).
    The DoubleRow pack dimension carries a hi/lo split of x:
    slot 0 = e4m3(x) against w, slot 1 = e4m3(16*(x - hi)) against w/16,
    so one DoubleRow matmul accumulates hi*w + lo*w/16 ~= x*w at ~bf16+
    precision into fp32 PSUM, at the bf16 cycle count. sign(W) is +-1 and
    w/16 is +-2^-4 - both exact in e4m3.
  - Host prep: x shard pre-tiled+packed into per-block, per-ko contiguous
    chunks [ko][ki=128, j=2, ns, p] fp8 so every DMA read segment is one
    contiguous run per partition AND the first matmul group of a block
    only waits on a quarter of the block's bytes. Output is written bf16
    and upcast to fp32 on host.
  - Device: per block, one x DMA per ko (sync/SP HWDGE ring), 4
    accumulating DoubleRow matmuls per 128-row subtile (lhsT = x pack
    [128,2,128], rhs = w pack [128,2,512], PSUM [128 n, 512 o]),
    bias-add on DVE copying PSUM -> SBUF bf16, one out-DMA per block on
    the scalar/ACT HWDGE ring (separate ring from reads).
  - n-assignment interleaved (lhsT column p of subtile s covers row
    p*n_sub + s) so each partition's output rows are consecutive ->
    one contiguous DRAM write segment per partition per block.
  - Block sizes ramp at start/end to shorten pipeline fill/drain; ~40
    dependency-free warmup matmuls run during the DMA fill to start the
    PE HAM clock-gate ramp early.
"""

import numpy as np
import ml_dtypes

import concourse.mybir as mybir
from concourse import bacc, bass_utils
from concourse.tile import TileContext

N_CORES = 8
N_TOTAL = 262144
IN_F = 512
OUT_F = 512
N_SHARD = N_TOTAL // N_CORES  # 32768
K_BLOCKS = IN_F // 128        # 4
P = 128
J = 2                         # DoubleRow pack: hi/lo

# ramped block schedule (rows per block); sums to N_SHARD
BLOCKS = [256, 256, 512] + [1024] * 30 + [512, 256, 256]
assert sum(BLOCKS) == N_SHARD

SPLIT_KO = True  # one x-DMA per ko block (finer matmul-ready granularity)

_nc_cache = None


def _build_nc():
    nc = bacc.Bacc(
        "TRN2", target_bir_lowering=False, debug=False, num_devices=N_CORES
    )
    # x pre-packed on host: per block, per ko a contiguous [128, 2*blk] chunk
    xt_d = nc.dram_tensor(
        "xt", [N_SHARD * IN_F * J], mybir.dt.float8e4, kind="ExternalInput"
    ).ap()
    wt_d = nc.dram_tensor(
        "wt", [P, K_BLOCKS, J, OUT_F], mybir.dt.float8e4, kind="ExternalInput"
    ).ap()
    b_d = nc.dram_tensor(
        "bias_bcast", [P, OUT_F], mybir.dt.float32, kind="ExternalInput"
    ).ap()
    out_d = nc.dram_tensor(
        "out", [N_SHARD, OUT_F], mybir.dt.bfloat16, kind="ExternalOutput"
    ).ap()

    with TileContext(nc) as tc:
        with (
            tc.tile_pool(name="const", bufs=1) as cpool,
            tc.tile_pool(name="xin", bufs=4) as xpool,
            tc.tile_pool(name="outp", bufs=4) as opool,
            tc.tile_pool(name="psum", bufs=7, space="PSUM") as ppool,
            tc.tile_pool(name="warm", bufs=1, space="PSUM") as wpool,
        ):
            # dependency-free dummy matmuls on a zeroed SBUF tile: they
            # schedule at engine boot and hold the PE busy so the HAM
            # clock-gate ramp starts before the first real matmul
            scratch = cpool.tile([P, P], mybir.dt.bfloat16)
            nc.gpsimd.memset(scratch[:], 0.0)
            wps = wpool.tile([P, 64], mybir.dt.float32)
            for _ in range(40):
                nc.tensor.matmul(
                    wps[:], lhsT=scratch[:], rhs=scratch[:, :64],
                    start=True, stop=True,
                )

            # constants on the ACT (write) ring so the first x-block
            # read isn't queued behind them on the SP ring
            wt_sb = cpool.tile([P, K_BLOCKS, J, OUT_F], mybir.dt.float8e4)
            nc.scalar.dma_start(wt_sb[:], wt_d[:])
            b_sb = cpool.tile([P, OUT_F], mybir.dt.float32)
            nc.scalar.dma_start(b_sb[:], b_d[:])

            off = 0
            for bi, blk in enumerate(BLOCKS):
                n_sub = blk // P
                x_sb = [
                    xpool.tile([P, J, n_sub, P], mybir.dt.float8e4,
                               tag=f"x{ko}", name=f"x{ko}")
                    for ko in range(K_BLOCKS)
                ]
                base = off * IN_F * J
                ko_sz = blk * P * J  # elements per ko chunk
                for ko in range(K_BLOCKS):
                    src = xt_d[
                        base + ko * ko_sz:base + (ko + 1) * ko_sz
                    ].rearrange("(ki f) -> ki f", ki=P)
                    nc.sync.dma_start(
                        x_sb[ko][:].rearrange("p j s q -> p (j s q)"), src
                    )
                o_sb = opool.tile([P, n_sub, OUT_F], mybir.dt.bfloat16)
                # rows [off, off+blk) as [p, s, o]: row = off + p*n_sub + s
                # -> contiguous (s, o) run per partition
                dst = out_d[off:off + blk, :].rearrange(
                    "(p s) o -> p s o", s=n_sub
                )
                # write each block in halves so the first half's out-DMA
                # overlaps the second half's matmuls
                h = max(1, n_sub // 2)
                for half in range((n_sub + h - 1) // h):
                    s0, s1 = half * h, min((half + 1) * h, n_sub)
                    for ns in range(s0, s1):
                        ps = ppool.tile([P, OUT_F], mybir.dt.float32)
                        for ko in range(K_BLOCKS):
                            # column p covers row off + p*n_sub + ns
                            nc.tensor.matmul(
                                ps[:],
                                lhsT=x_sb[ko][:, :, ns, :],
                                rhs=wt_sb[:, ko, :, :],
                                start=(ko == 0),
                                stop=(ko == K_BLOCKS - 1),
                                perf_mode=mybir.MatmulPerfMode.DoubleRow,
                            )
                        nc.vector.tensor_add(o_sb[:, ns, :], ps[:], b_sb[:])
                    nc.scalar.dma_start(
                        dst[:, s0:s1, :], o_sb[:, s0:s1, :]
                    )
                off += blk

    nc.finalize()
    return nc


_E4 = ml_dtypes.float8_e4m3


def _pack_x_shard(shard_f32: np.ndarray) -> np.ndarray:
    """[N_SHARD, 512] fp32 -> flat fp8 per-block [ko][ki, j, ns, p] pack."""
    chunks = []
    off = 0
    for blk in BLOCKS:
        n_sub = blk // P
        b = shard_f32[off:off + blk, :].reshape(P, n_sub, K_BLOCKS, P)
        # axes: [p, ns, ko, ki]
        hi = b.astype(_E4)
        lo = ((b - hi.astype(np.float32)) * 16.0).astype(_E4)
        pack = np.stack([hi, lo], axis=0)        # [j, p, ns, ko, ki]
        pack = pack.transpose(3, 4, 0, 2, 1)     # [ko, ki, j, ns, p]
        chunks.append(np.ascontiguousarray(pack).reshape(-1))
        off += blk
    return np.concatenate(chunks)


def kernel(x: np.ndarray, weight: np.ndarray, bias: np.ndarray, **run_kwargs):
    global _nc_cache
    if _nc_cache is None:
        _nc_cache = _build_nc()
    nc = _nc_cache

    x = np.asarray(x)
    weight = np.asarray(weight)
    bias = np.asarray(bias)

    wb = np.sign(weight.astype(np.float32)).T          # [512 i, 512 o]
    wbr = wb.reshape(K_BLOCKS, P, OUT_F)               # [ko, ki, o]
    wt = np.stack(
        [wbr.astype(_E4), (wbr / 16.0).astype(_E4)], axis=2
    )                                                  # [ko, ki, j, o]
    wt = np.ascontiguousarray(wt.transpose(1, 0, 2, 3))  # [ki, ko, j, o]
    bias_bcast = np.ascontiguousarray(
        np.broadcast_to(bias.astype(np.float32)[None, :], (P, OUT_F))
    )

    in_maps = []
    for c in range(N_CORES):
        shard = np.ascontiguousarray(
            x[c * N_SHARD:(c + 1) * N_SHARD, :], dtype=np.float32
        )
        in_maps.append(
            {"xt": _pack_x_shard(shard), "wt": wt, "bias_bcast": bias_bcast}
        )

    res = bass_utils.run_bass_kernel_spmd(
        nc, in_maps, core_ids=list(range(N_CORES)), **run_kwargs
    )
    out = np.empty((N_TOTAL, OUT_F), dtype=np.float32)
    for c in range(N_CORES):
        out[c * N_SHARD:(c + 1) * N_SHARD, :] = res.results[c]["out"].astype(
            np.float32
        )
    if run_kwargs:
        kernel.last_result = res
    return out
